# revision 1
# baseline (speedup 1.0000x reference)
import sys

if "/opt/trn_rl_repo" not in sys.path:
    sys.path.insert(0, "/opt/trn_rl_repo")

import numpy as np

from concourse import bacc, bass_utils, tile
from concourse.bass import IndirectOffsetOnAxis, mybir
from concourse.masks import make_identity

f32 = mybir.dt.float32
i32 = mybir.dt.int32
i16 = mybir.dt.int16
Alu = mybir.AluOpType
Act = mybir.ActivationFunctionType
AX = mybir.AxisListType

R = 8388608
NCORES = 8
RC = R // NCORES          # rows per core
P = 128
F = RC // P               # 8192 free elems per partition
NCHUNK = 4
FC = F // NCHUNK          # 2048
CAP = 64                  # per-core per-class candidate capacity
NS = 384                  # merged NMS problem size (per class, <=512)
NB = NS // P              # 3 row blocks
HALF = 200
SIGMA = 10.0
IOU_TH = 0.7
VALID_TH = -1.0e8
T_JAC = 4
SQRT5 = 5.0 ** 0.5

_CACHE = {}
LAST_RESULTS = None


def _program(nc, tc, ct_t, cls_t, lp_t, lt_t, anc_t, rinit_t, out_t):
    dve = nc.vector
    gps = nc.gpsimd
    act = nc.scalar
    pe = nc.tensor

    with tc.tile_pool(name="sb", bufs=1) as sb, \
         tc.tile_pool(name="io", bufs=4) as io, \
         tc.tile_pool(name="pp", bufs=1, space="PSUM") as pp, \
         tc.tile_pool(name="dr", bufs=1, space="DRAM") as dr:

        def S(name, shape, dtype=f32):
            return sb.tile(shape, dtype, name=name, tag=name)

        def SC(tag, shape=None, bufs=2):
            return sb.tile(shape or [P, NS], f32, name=tag, tag=tag, bufs=bufs)

        # ---------- constants ----------
        ident = S("ident", [P, P])
        make_identity(nc, ident)
        ones1 = S("ones1", [1, P])
        dve.memset(ones1, 1.0)
        ones11 = ones1[0:1, 0:1]
        onesrow = S("onesrow", [1, NS])
        dve.memset(onesrow, 1.0)
        colid = S("colid", [P, NS])
        gps.iota(colid, pattern=[[1, NS]], base=0, channel_multiplier=0,
                 allow_small_or_imprecise_dtypes=True)
        pcol = S("pcol", [P, 1])
        gps.iota(pcol, pattern=[[0, 1]], base=0, channel_multiplier=1,
                 allow_small_or_imprecise_dtypes=True)
        p8192 = S("p8192", [P, 1])
        gps.iota(p8192, pattern=[[0, 1]], base=0, channel_multiplier=F,
                 allow_small_or_imprecise_dtypes=True)
        s8 = S("s8", [P, 8])
        gps.iota(s8, pattern=[[1, 8]], base=0, channel_multiplier=0,
                 allow_small_or_imprecise_dtypes=True)
        dumpr = S("dumpr", [P, 1])
        gps.iota(dumpr, pattern=[[0, 1]], base=2 * CAP, channel_multiplier=1,
                 allow_small_or_imprecise_dtypes=True)
        biota16 = S("biota16", [P, F], i16)
        gps.iota(biota16, pattern=[[-1, F]], base=0, channel_multiplier=0)
        UT = S("UT", [P, P])
        dve.tensor_scalar(out=UT, in0=colid[:, 0:P], scalar1=pcol, scalar2=None,
                          op0=Alu.is_gt)
        JM = S("JM", [P, NS])
        dve.tensor_scalar(out=JM, in0=colid, scalar1=pcol, scalar2=None,
                          op0=Alu.is_gt)

        # ---------- phase 1: scan + top-8 extract per partition ----------
        # v16[p,c] = -(min(ct,2)*8192 + c)  (ct pre-clamped on host)
        # descending top-8: negatives (ct=0) first, then positives, then invalid
        v16 = S("v16", [P, F], i16)
        for c in range(NCHUNK):
            sl = slice(c * FC, (c + 1) * FC)
            ctc = io.tile([P, FC], i16, name=f"ctc{c}", tag="ctc", bufs=4)
            gps.dma_start(ctc, ct_t.ap()[:, sl])
            dve.scalar_tensor_tensor(out=v16[:, sl], in0=ctc, scalar=-float(F),
                                     in1=biota16[:, sl], op0=Alu.mult, op1=Alu.add)
        v8_16 = S("v8_16", [P, 8], i16)
        dve.max(v8_16, v16)
        v8 = S("v8", [P, 8])
        dve.tensor_copy(v8, v8_16)

        # decode: neg class v in (-8192, 0]; pos in (-16384, -8192]; invalid <= -16384
        isneg = S("isneg", [P, 8])
        dve.tensor_scalar(out=isneg, in0=v8, scalar1=-(float(F) - 0.5), scalar2=None,
                          op0=Alu.is_gt)
        validm = S("validm", [P, 8])
        dve.tensor_scalar(out=validm, in0=v8, scalar1=-(2.0 * F - 0.5), scalar2=None,
                          op0=Alu.is_gt)
        ispos = S("ispos", [P, 8])
        dve.tensor_tensor(out=ispos, in0=validm, in1=isneg, op=Alu.subtract)
        # col c = -v - ispos*8192 ; local row idx = c + 8192*p  (0 for invalid)
        negv = S("negv", [P, 8])
        dve.tensor_scalar(out=negv, in0=v8, scalar1=-1.0, scalar2=None, op0=Alu.mult)
        cbase = S("cbase", [P, 8])
        dve.tensor_scalar(out=cbase, in0=ispos, scalar1=float(F), scalar2=None,
                          op0=Alu.mult)
        i_c = S("i_c", [P, 8])
        dve.tensor_tensor(out=i_c, in0=negv, in1=cbase, op=Alu.subtract)
        i_loc = S("i_loc", [P, 8])
        dve.tensor_scalar(out=i_loc, in0=i_c, scalar1=p8192, scalar2=None, op0=Alu.add)
        i_s = S("i_s", [P, 8])
        dve.tensor_tensor(out=i_s, in0=i_loc, in1=validm, op=Alu.mult)
        idx32 = S("idx32", [P, 8], i32)
        dve.tensor_copy(idx32, i_s)

        # ---------- gathers ----------
        Gc = S("Gc", [P, 8, 2])
        Gp = S("Gp", [P, 8, 2])
        Gt = S("Gt", [P, 8, 2])
        Ga = S("Ga", [P, 8, 4])
        for s in range(8):
            off = IndirectOffsetOnAxis(ap=idx32[:, s:s + 1], axis=0)
            gps.indirect_dma_start(out=Gc[:, s, :], out_offset=None,
                                   in_=cls_t.ap(), in_offset=off)
            gps.indirect_dma_start(out=Gp[:, s, :], out_offset=None,
                                   in_=lp_t.ap(), in_offset=off)
            gps.indirect_dma_start(out=Gt[:, s, :], out_offset=None,
                                   in_=lt_t.ap(), in_offset=off)
            gps.indirect_dma_start(out=Ga[:, s, :], out_offset=None,
                                   in_=anc_t.ap(), in_offset=off)

        # ---------- per-candidate losses ----------
        # ce = softplus((1-2*ispos) * (logit1 - logit0))
        dba = S("dba", [P, 8])
        dve.tensor_tensor(out=dba, in0=Gc[:, :, 1], in1=Gc[:, :, 0], op=Alu.subtract)
        sfac = S("sfac", [P, 8])
        dve.tensor_scalar(out=sfac, in0=ispos, scalar1=-2.0, scalar2=1.0,
                          op0=Alu.mult, op1=Alu.add)
        zz = S("zz", [P, 8])
        dve.tensor_tensor(out=zz, in0=dba, in1=sfac, op=Alu.mult)
        # softplus(z) = relu(z) + ln(1 + exp(-|z|))  (Softplus has no ACT table)
        az = S("az", [P, 8])
        act.activation(out=az, in_=zz, func=Act.Abs)
        enz = S("enz", [P, 8])
        act.activation(out=enz, in_=az, func=Act.Exp, scale=-1.0)
        ep1 = S("ep1", [P, 8])
        dve.tensor_scalar(out=ep1, in0=enz, scalar1=1.0, scalar2=None, op0=Alu.add)
        lg = S("lg", [P, 8])
        act.activation(out=lg, in_=ep1, func=Act.Ln)
        rz = S("rz", [P, 8])
        act.activation(out=rz, in_=zz, func=Act.Relu)
        cet = S("cet", [P, 8])
        dve.tensor_tensor(out=cet, in0=rz, in1=lg, op=Alu.add)
        # smooth L1: per coord m=min(|d|,1/sigma); 0.5*sigma*m^2 + (|d|-m)
        dd = S("dd", [P, 8, 2])
        dve.tensor_tensor(out=dd, in0=Gt, in1=Gp, op=Alu.subtract)
        ad = S("ad", [P, 8, 2])
        act.activation(out=ad, in_=dd, func=Act.Abs)
        mm = S("mm", [P, 8, 2])
        dve.tensor_scalar(out=mm, in0=ad, scalar1=1.0 / SIGMA, scalar2=None,
                          op0=Alu.min)
        qq = S("qq", [P, 8, 2])
        dve.tensor_tensor(out=qq, in0=ad, in1=mm, op=Alu.subtract)
        sq = S("sq", [P, 8, 2])
        act.activation(out=sq, in_=mm, func=Act.Square, scale=(0.5 * SIGMA) ** 0.5)
        slc = S("slc", [P, 8, 2])
        dve.tensor_tensor(out=slc, in0=sq, in1=qq, op=Alu.add)
        sl1v = S("sl1v", [P, 8])
        dve.tensor_tensor(out=sl1v, in0=slc[:, :, 0], in1=slc[:, :, 1], op=Alu.add)

        # records [key, ce, sl1, x1, y1, x2, y2, 0]
        rec = S("rec", [P, 8, 8])
        dve.memset(rec, 0.0)
        ksl = S("ksl", [P, 8])
        dve.tensor_tensor(out=ksl, in0=sl1v, in1=ispos, op=Alu.mult)
        dve.tensor_tensor(out=rec[:, :, 0], in0=cet, in1=ksl, op=Alu.add)
        dve.tensor_copy(rec[:, :, 1], cet)
        dve.tensor_copy(rec[:, :, 2], sl1v)
        dve.tensor_copy(rec[:, :, 3:7], Ga)

        # ---------- compaction: scatter records to per-class DRAM rows ----------
        cntn = S("cntn", [P, 1])
        dve.tensor_reduce(out=cntn, in_=isneg, axis=AX.X, op=Alu.add)
        cntv = S("cntv", [P, 1])
        dve.tensor_reduce(out=cntv, in_=validm, axis=AX.X, op=Alu.add)
        cntp = S("cntp", [P, 1])
        dve.tensor_tensor(out=cntp, in0=cntv, in1=cntn, op=Alu.subtract)
        counts2 = S("counts2", [P, 2])
        dve.tensor_copy(counts2[:, 0:1], cntn)
        dve.tensor_copy(counts2[:, 1:2], cntp)
        offs_ps = pp.tile([P, 512], f32, name="offs_ps", tag="mm_ps", bufs=2)
        pe.matmul(offs_ps[:, 0:2], lhsT=UT, rhs=counts2, start=True, stop=True)
        offs = S("offs", [P, 2])
        dve.tensor_copy(offs, offs_ps[:, 0:2])
        # target row: neg -> offs_n + s ; pos -> 64 + offs_p + (s - cntn); invalid -> 128+p
        tcn = S("tcn", [P, 8])
        dve.tensor_scalar(out=tcn, in0=ispos, scalar1=cntn, scalar2=None, op0=Alu.mult)
        jj = S("jj", [P, 8])
        dve.tensor_tensor(out=jj, in0=s8, in1=tcn, op=Alu.subtract)
        opn64 = S("opn64", [P, 1])
        dve.tensor_tensor(out=opn64, in0=offs[:, 1:2], in1=offs[:, 0:1],
                          op=Alu.subtract)
        dve.tensor_scalar(out=opn64, in0=opn64, scalar1=float(CAP), scalar2=None,
                          op0=Alu.add)
        t1 = S("t1", [P, 8])
        dve.tensor_scalar(out=t1, in0=ispos, scalar1=opn64, scalar2=None, op0=Alu.mult)
        j2 = S("j2", [P, 8])
        dve.tensor_scalar(out=j2, in0=jj, scalar1=offs[:, 0:1], scalar2=None,
                          op0=Alu.add)
        sidxf = S("sidxf", [P, 8])
        dve.tensor_tensor(out=sidxf, in0=j2, in1=t1, op=Alu.add)
        d8 = S("d8", [P, 8])
        dve.tensor_scalar(out=d8, in0=sidxf, scalar1=dumpr, scalar2=None,
                          op0=Alu.subtract)
        d8v = S("d8v", [P, 8])
        dve.tensor_tensor(out=d8v, in0=d8, in1=validm, op=Alu.mult)
        sidxf2 = S("sidxf2", [P, 8])
        dve.tensor_scalar(out=sidxf2, in0=d8v, scalar1=dumpr, scalar2=None,
                          op0=Alu.add)
        sidx32 = S("sidx32", [P, 8], i32)
        dve.tensor_copy(sidx32, sidxf2)

        rec_out = dr.tile([2 * P, 8], f32, name="rec_out", tag="rec_out")
        gps.dma_start(rec_out[:, :], rinit_t.ap())
        for s in range(8):
            gps.indirect_dma_start(
                out=rec_out[:, :],
                out_offset=IndirectOffsetOnAxis(ap=sidx32[:, s:s + 1], axis=0),
                in_=rec[:, s, :], in_offset=None)

        # ---------- all-gather ----------
        merged = dr.tile([NCORES * 2 * P, 8], f32, name="merged", tag="merged")
        gps.collective_compute(
            "AllGather", Alu.bypass,
            replica_groups=[list(range(NCORES))],
            ins=[rec_out.opt()], outs=[merged.opt()])

        # ---------- per-class merge + sort + NMS (replicated) ----------
        cls_scal = {}
        for ci, cname in ((0, "n"), (1, "p")):
            # load 512 candidate records as [128 part x 4 blocks x 8 fields]
            crec = S(f"crec_{cname}", [P, 32])
            for k in range(NCORES):
                rs = k * 2 * P + ci * CAP
                dve_part = (k % 2) * CAP
                blk = k // 2
                gps.dma_start(
                    crec[dve_part:dve_part + CAP, blk * 8:(blk + 1) * 8],
                    merged[rs:rs + CAP, :])
            # candidate j = q*128 + p holds fields crec[p, q*8: q*8+8]
            # keys of all 512 candidates to one row via PE transpose per block
            keyrows = S(f"keyrows_{cname}", [1, 512])
            for q in range(4):
                ktp = pp.tile([1, 512], f32, name=f"ktp_{cname}{q}", tag="sp_ps",
                              bufs=3)
                pe.matmul(ktp[0:1, 0:P], lhsT=crec[:, q * 8:q * 8 + 1], rhs=ident,
                          start=True, stop=True)
                dve.tensor_copy(keyrows[0:1, P * q:P * (q + 1)], ktp[0:1, 0:P])
            keyB_ps = pp.tile([P, 512], f32, name=f"keyB_ps_{cname}", tag="mm_ps",
                              bufs=2)
            for q in range(4):
                pe.matmul(keyB_ps[:, P * q:P * (q + 1)], lhsT=ones1,
                          rhs=keyrows[0:1, P * q:P * (q + 1)], start=True, stop=True)
            keyB = S(f"keyB_{cname}", [P, 512])
            act.activation(out=keyB, in_=keyB_ps, func=Act.Copy)
            # rank = count of keys strictly greater (keys are unique)
            ranks = S(f"ranks_{cname}", [P, 4])
            rtrash = sb.tile([P, 512], f32, name="rtrash", tag="rtrash", bufs=2)
            for q in range(4):
                dve.tensor_scalar(out=rtrash, in0=keyB,
                                  scalar1=crec[:, q * 8:q * 8 + 1], scalar2=0.0,
                                  op0=Alu.is_gt, op1=Alu.add,
                                  accum_out=ranks[:, q:q + 1])
            ranks32 = S(f"ranks32_{cname}", [P, 4], i32)
            dve.tensor_copy(ranks32, ranks)
            # scatter records to DRAM row = rank (descending sort)
            sortbuf = dr.tile([512, 8], f32, name=f"sortbuf_{cname}",
                              tag=f"sortbuf_{cname}")
            for q in range(4):
                gps.indirect_dma_start(
                    out=sortbuf[:, :],
                    out_offset=IndirectOffsetOnAxis(ap=ranks32[:, q:q + 1], axis=0),
                    in_=crec[:, q * 8:q * 8 + 8], in_offset=None)
            # reload the top NS sorted records: candidate s = b*128 + p
            srt = S(f"srt_{cname}", [P, NB * 8])
            for b in range(NB):
                gps.dma_start(srt[:, b * 8:(b + 1) * 8],
                              sortbuf[b * P:(b + 1) * P, :])
            # per-field rows of the sorted candidates via strided DRAM reads
            srow = {}
            for fi, fname in ((0, "key"), (1, "ce"), (2, "sl"), (3, "x1"),
                              (4, "y1"), (5, "x2"), (6, "y2")):
                if fi == 2 and ci == 0:
                    continue
                fr = S(f"srow_{cname}_{fname}", [1, NS])
                gps.dma_start(fr, sortbuf[0:NS, fi:fi + 1].rearrange("a b -> b a"))
                srow[fname] = fr
            validrow = S(f"validrow_{cname}", [1, NS])
            dve.tensor_scalar(out=validrow, in0=srow["key"], scalar1=VALID_TH,
                              scalar2=None, op0=Alu.is_gt)
            cerow = srow["ce"]
            sl1row = srow.get("sl")

            # broadcast box fields of all NS candidates to every partition
            fieldB = {}
            for fname in ("x1", "y1", "x2", "y2"):
                frow = srow[fname]
                bb = pp.tile([P, 512], f32, name=f"bb_{cname}_{fname}", tag="mm_ps",
                             bufs=2)
                for b in range(NB):
                    pe.matmul(bb[:, P * b:P * (b + 1)], lhsT=ones1,
                              rhs=frow[0:1, P * b:P * (b + 1)],
                              start=True, stop=True)
                fB = S(f"{fname}B_{cname}", [P, NS])
                act.activation(out=fB, in_=bb[:, 0:NS], func=Act.Copy)
                fieldB[fname] = fB
            x1B, y1B, x2B, y2B = (fieldB[n] for n in ("x1", "y1", "x2", "y2"))
            wid = SC("sc_wid")
            dve.tensor_tensor(out=wid, in0=x2B, in1=x1B, op=Alu.subtract)
            hei = SC("sc_hei")
            gps.tensor_tensor(out=hei, in0=y2B, in1=y1B, op=Alu.subtract)
            areaB = S(f"areaB_{cname}", [P, NS])
            dve.tensor_tensor(out=areaB, in0=wid, in1=hei, op=Alu.mult)
            # per-partition areas of candidate i = b*128+p
            areai = S(f"areai_{cname}", [P, NB])
            tmpa = S(f"tmpa_{cname}", [P, 2])
            for b in range(NB):
                dve.tensor_tensor(out=tmpa[:, 0:1], in0=srt[:, b * 8 + 5:b * 8 + 6],
                                  in1=srt[:, b * 8 + 3:b * 8 + 4], op=Alu.subtract)
                dve.tensor_tensor(out=tmpa[:, 1:2], in0=srt[:, b * 8 + 6:b * 8 + 7],
                                  in1=srt[:, b * 8 + 4:b * 8 + 5], op=Alu.subtract)
                dve.tensor_tensor(out=areai[:, b:b + 1], in0=tmpa[:, 0:1],
                                  in1=tmpa[:, 1:2], op=Alu.mult)

            # suppression matrix blocks: rows i = b*128+p, cols j = b*128+c
            Ms = []
            for b in range(NB):
                w = NS - P * b
                jsl = slice(P * b, NS)
                x1i = srt[:, b * 8 + 3:b * 8 + 4]
                y1i = srt[:, b * 8 + 4:b * 8 + 5]
                x2i = srt[:, b * 8 + 5:b * 8 + 6]
                y2i = srt[:, b * 8 + 6:b * 8 + 7]
                xx1 = SC("sc_xx1")
                dve.tensor_scalar(out=xx1[:, :w], in0=x1B[:, jsl], scalar1=x1i,
                                  scalar2=None, op0=Alu.max)
                yy1 = SC("sc_yy1")
                gps.tensor_scalar(out=yy1[:, :w], in0=y1B[:, jsl], scalar1=y1i,
                                  scalar2=None, op0=Alu.max)
                xx2 = SC("sc_xx2")
                dve.tensor_scalar(out=xx2[:, :w], in0=x2B[:, jsl], scalar1=x2i,
                                  scalar2=None, op0=Alu.min)
                yy2 = SC("sc_yy2")
                gps.tensor_scalar(out=yy2[:, :w], in0=y2B[:, jsl], scalar1=y2i,
                                  scalar2=None, op0=Alu.min)
                dxx = SC("sc_dx")
                dve.tensor_tensor(out=dxx[:, :w], in0=xx2[:, :w], in1=xx1[:, :w],
                                  op=Alu.subtract)
                dyy = SC("sc_dy")
                gps.tensor_tensor(out=dyy[:, :w], in0=yy2[:, :w], in1=yy1[:, :w],
                                  op=Alu.subtract)
                dxr = SC("sc_dxr")
                act.activation(out=dxr[:, :w], in_=dxx[:, :w], func=Act.Relu)
                dyr = SC("sc_dyr")
                act.activation(out=dyr[:, :w], in_=dyy[:, :w], func=Act.Relu)
                inter = SC("sc_int")
                dve.tensor_tensor(out=inter[:, :w], in0=dxr[:, :w], in1=dyr[:, :w],
                                  op=Alu.mult)
                # iou > th  <=>  (1+th)*inter > th*(area_i + area_j)
                rhsu = SC("sc_rhs")
                gps.tensor_scalar(out=rhsu[:, :w], in0=areaB[:, jsl],
                                  scalar1=areai[:, b:b + 1], scalar2=IOU_TH,
                                  op0=Alu.add, op1=Alu.mult)
                mraw = SC("sc_mraw")
                dve.scalar_tensor_tensor(out=mraw[:, :w], in0=inter[:, :w],
                                         scalar=1.0 + IOU_TH, in1=rhsu[:, :w],
                                         op0=Alu.mult, op1=Alu.is_gt)
                Mb = sb.tile([P, w], f32, name=f"M{b}", tag=f"M{b}", bufs=2)
                gps.tensor_tensor(out=Mb, in0=mraw[:, :w], in1=JM[:, :w], op=Alu.mult)
                Ms.append(Mb)

            # Jacobi NMS iterations (fixpoint == sequential NMS)
            keepcol = S(f"keepcol_{cname}", [P, 4])
            vc_ps = pp.tile([P, 8], f32, name=f"vc_{cname}", tag="col_ps", bufs=1)
            for b in range(NB):
                pe.matmul(vc_ps[:, 2 * b:2 * b + 1],
                          lhsT=validrow[0:1, P * b:P * (b + 1)], rhs=ones11,
                          start=True, stop=True)
            for b in range(NB):
                dve.tensor_copy(keepcol[:, b:b + 1], vc_ps[:, 2 * b:2 * b + 1])
            keeprow = S(f"keeprow_{cname}", [1, NS])
            suprow = S(f"suprow_{cname}", [1, NS])
            for t in range(T_JAC):
                sps = []
                for b in range(NB):
                    w = NS - P * b
                    sp = pp.tile([1, 512], f32, name=f"sp{b}", tag="sp_ps", bufs=3)
                    pe.matmul(sp[0:1, 0:w], lhsT=keepcol[:, b:b + 1],
                              rhs=Ms[b][:, 0:w], start=True, stop=True)
                    sps.append(sp)
                dve.tensor_copy(suprow, sps[0][0:1, 0:NS])
                dve.tensor_tensor(out=suprow[0:1, P:NS], in0=suprow[0:1, P:NS],
                                  in1=sps[1][0:1, 0:NS - P], op=Alu.add)
                dve.tensor_tensor(out=suprow[0:1, 2 * P:NS], in0=suprow[0:1, 2 * P:NS],
                                  in1=sps[2][0:1, 0:NS - 2 * P], op=Alu.add)
                dve.scalar_tensor_tensor(out=keeprow, in0=suprow, scalar=0.5,
                                         in1=validrow, op0=Alu.is_lt, op1=Alu.mult)
                if t < T_JAC - 1:
                    kc_ps = pp.tile([P, 8], f32, name=f"kc_{cname}_{t}",
                                    tag="col_ps", bufs=1)
                    for b in range(NB):
                        pe.matmul(kc_ps[:, 2 * b:2 * b + 1],
                                  lhsT=keeprow[0:1, P * b:P * (b + 1)], rhs=ones11,
                                  start=True, stop=True)
                    for b in range(NB):
                        dve.tensor_copy(keepcol[:, b:b + 1], kc_ps[:, 2 * b:2 * b + 1])

            # selection: kept and rank-within-kept < HALF
            cums = S(f"cums_{cname}", [1, NS])
            dve.tensor_tensor_scan(out=cums, data0=onesrow, data1=keeprow,
                                   initial=0.0, op0=Alu.mult, op1=Alu.add)
            selrow = S(f"selrow_{cname}", [1, NS])
            dve.scalar_tensor_tensor(out=selrow, in0=cums, scalar=HALF + 0.5,
                                     in1=keeprow, op0=Alu.is_le, op1=Alu.mult)
            strash = sb.tile([1, NS], f32, name="strash", tag="strash", bufs=2)

            def dot_sum(name, rowA, rowB):
                out = S(name, [1, 1])
                dve.scalar_tensor_tensor(out=strash, in0=rowA, scalar=1.0, in1=rowB,
                                         op0=Alu.mult, op1=Alu.mult, accum_out=out)
                return out

            sc = {}
            sc["selce"] = dot_sum(f"selce_{cname}", selrow, cerow)
            sc["valce"] = dot_sum(f"valce_{cname}", validrow, cerow)
            if ci == 0:
                nv = S(f"nv_{cname}", [1, 1])
                dve.tensor_reduce(out=nv, in_=validrow, axis=AX.X, op=Alu.add)
                sc["nv"] = nv
            if ci == 1:
                sc["selsl"] = dot_sum(f"selsl_{cname}", selrow, sl1row)
                sc["valsl"] = dot_sum(f"valsl_{cname}", validrow, sl1row)
                nk = S(f"nk_{cname}", [1, 1])
                dve.tensor_reduce(out=nk, in_=keeprow, axis=AX.X, op=Alu.add)
                sc["nk"] = nk
            cls_scal[cname] = sc

        # ---------- final scalar assembly ----------
        def s1(name):
            return S(name, [1, 1])

        def blend(name, full, sel, trunc):
            dif = s1(name + "_d")
            dve.tensor_tensor(out=dif, in0=sel, in1=full, op=Alu.subtract)
            con = s1(name + "_c")
            dve.tensor_tensor(out=con, in0=trunc, in1=dif, op=Alu.mult)
            out = s1(name)
            dve.tensor_tensor(out=out, in0=full, in1=con, op=Alu.add)
            return out

        pn = cls_scal["p"]
        nn = cls_scal["n"]
        truncp = s1("truncp")
        dve.tensor_scalar(out=truncp, in0=pn["nk"], scalar1=HALF + 0.5, scalar2=None,
                          op0=Alu.is_gt)
        truncn = s1("truncn")
        dve.tensor_scalar(out=truncn, in0=nn["nv"], scalar1=HALF + 0.5, scalar2=None,
                          op0=Alu.is_gt)
        pos_cls = blend("pos_cls", pn["valce"], pn["selce"], truncp)
        pos_loc = blend("pos_loc", pn["valsl"], pn["selsl"], truncp)
        neg_cls = blend("neg_cls", nn["valce"], nn["selce"], truncn)
        keep_num = s1("keep_num")
        dve.tensor_scalar(out=keep_num, in0=pn["nk"], scalar1=float(HALF),
                          scalar2=None, op0=Alu.min)
        keep_num_neg = s1("keep_num_neg")
        dve.tensor_scalar(out=keep_num_neg, in0=nn["nv"], scalar1=float(HALF),
                          scalar2=None, op0=Alu.min)
        den = s1("den")
        dve.tensor_tensor(out=den, in0=keep_num, in1=keep_num_neg, op=Alu.add)
        rden = s1("rden")
        dve.reciprocal(rden, den)
        csum = s1("csum")
        dve.tensor_tensor(out=csum, in0=neg_cls, in1=pos_cls, op=Alu.add)
        rkn = s1("rkn")
        dve.reciprocal(rkn, keep_num)
        outsb = S("outsb", [1, 2])
        dve.tensor_tensor(out=outsb[0:1, 0:1], in0=csum, in1=rden, op=Alu.mult)
        dve.tensor_tensor(out=outsb[0:1, 1:2], in0=pos_loc, in1=rkn, op=Alu.mult)
        gps.dma_start(out_t.ap(), outsb)


def _build():
    nc = bacc.Bacc("TRN2", target_bir_lowering=False, debug=False,
                   num_devices=NCORES)
    ct_t = nc.dram_tensor("ct", [P, F], i16, kind="ExternalInput")
    cls_t = nc.dram_tensor("cls", [RC, 2], f32, kind="ExternalInput")
    lp_t = nc.dram_tensor("lp", [RC, 2], f32, kind="ExternalInput")
    lt_t = nc.dram_tensor("lt", [RC, 2], f32, kind="ExternalInput")
    anc_t = nc.dram_tensor("anc", [RC, 4], f32, kind="ExternalInput")
    rinit_t = nc.dram_tensor("rinit", [2 * P, 8], f32, kind="ExternalInput")
    out_t = nc.dram_tensor("out_loss", [1, 2], f32, kind="ExternalOutput")
    with tile.TileContext(nc) as tc:
        _program(nc, tc, ct_t, cls_t, lp_t, lt_t, anc_t, rinit_t, out_t)
    nc.compile()
    return nc


def _get_nc():
    if "nc" not in _CACHE:
        _CACHE["nc"] = _build()
    return _CACHE["nc"]


def kernel(**inputs):
    global LAST_RESULTS
    nc = _get_nc()
    ct = np.minimum(np.asarray(inputs["cls_target"]).reshape(R), 2).astype(np.int16)
    cp = np.asarray(inputs["cls_pred"], dtype=np.float32).reshape(R, 2)
    lp = np.asarray(inputs["loc_pred"], dtype=np.float32).reshape(R, 2)
    lt = np.asarray(inputs["loc_target"], dtype=np.float32).reshape(R, 2)
    an = np.asarray(inputs["anchors"], dtype=np.float32).reshape(R, 4)
    in_maps = []
    for k in range(NCORES):
        sl = slice(k * RC, (k + 1) * RC)
        rinit = np.zeros((2 * P, 8), np.float32)
        rinit[:, 0] = -(1.0e9 + (k * 2 * P + np.arange(2 * P)) * 4096.0)
        in_maps.append({
            "ct": np.ascontiguousarray(ct[sl].reshape(P, F)),
            "cls": np.ascontiguousarray(cp[sl]),
            "lp": np.ascontiguousarray(lp[sl]),
            "lt": np.ascontiguousarray(lt[sl]),
            "anc": np.ascontiguousarray(an[sl]),
            "rinit": rinit,
        })
    res = bass_utils.run_bass_kernel_spmd(nc, in_maps, list(range(NCORES)))
    LAST_RESULTS = res
    out = np.asarray(res.results[0]["out_loss"], dtype=np.float32).reshape(2)
    return (np.float32(out[0]), np.float32(out[1]))


if __name__ == "__main__":
    nc = _build()
    print("compile OK")

